# revision 23
# baseline (speedup 1.0000x reference)
"""LocallyConnected1D (B=8, L=4096, C=64, K=3, F=64) on 8 TRN2 NeuronCores.

out[b, l, f] = sum_{k,c} x[b, l+k, c] * kernel[l, k, c, f] + bias[l, f]

Strategy (spatial sharding, 512 output positions per core):
  - For each pair of adjacent output positions (l0+2i, l0+2i+1) build a
    block-diagonal stationary tile lhsT (128 x 16): partitions = 2 phases x 64
    channels, columns = 2 phases x 8 batch.  Streaming operand = the pair's
    per-position weights (128 x 64).  Three PSUM-accumulated matmuls per pair
    (one per tap k, using x-pair tiles shifted by k) produce out (16, 64).
  - Groups of 8 pairs are dispatched to 4 independent 32-column strips of the
    PE array (tile_position), each strip accumulating into its own PSUM bank.
  - HBM traffic is minimized (the kernel is HBM-bound): weights stream as
    bf16 blocks; x is DMA'd ONCE densely (0.5 MB) and the block-diagonal
    TE/TO stationary tiles are built on-chip with 4 strided DVE copies
    (zero quadrants pre-memset on GpSimd); outputs return as bf16.
  - Compute in bf16 (PSUM accumulation in f32); bias added on host.
"""

import numpy as np
import ml_dtypes

import concourse.bass as bass
import concourse.mybir as mybir
import concourse.tile as tile
from concourse import bacc
from concourse.bass import ds, ts
from concourse.bass_utils import run_bass_kernel_spmd

B, L, C, K, F = 8, 4096, 64, 3, 64
L_OUT = (L - K) + 1  # 4094
N_CORES = 8
P_CORE = 512          # output positions per core (last core: 510 real + 2 pad)
PAIRS = P_CORE // 2   # 256

# pairs per weight-DMA block; tapered tail lets the last blocks' compute and
# drains pipeline under the DMA stream, shrinking the post-stream tail
BLOCKS = [32, 32, 32, 32, 32, 24, 24, 16, 16, 8, 8]
assert sum(BLOCKS) == PAIRS and all(n % 8 == 0 for n in BLOCKS)
# pairs per compute chunk (one PSUM bank-full across up to 4 PE strips)
CCHUNK = 32
# output chunks (by block index): few out-DMAs, tiny final one
OUT_CHUNKS = [(0, 4), (4, 7), (7, 10), (10, 11)]
DRAIN_SPLIT = False  # alternate PSUM drains between DVE and Activation
OUT_MODE = "block_gpsimd"  # "chunk_act" | "block_gpsimd"
DUAL_QUEUE = True   # alternate weight blocks between the two HWDGE queues

DT = mybir.dt.bfloat16
NPDT = ml_dtypes.bfloat16
DT_PS = mybir.dt.float32

W_COLS = PAIRS * K * F           # 49152 weight cols (192 per pair)
X_COLS = (PAIRS + 1) * 8         # 2056 dense-x cols

_CACHE = {}


def _build_body(nc, wpool, pspool, xbufs, wd, xd, od):
    TEbuf, TObuf, xt, obuf = xbufs

    # dense x in on the Activation HWDGE queue (weights stream on SP's)
    nc.scalar.dma_start(xt[:], xd[:])

    # scatter dense x into the diagonal quadrants (strided DVE copies;
    # TO's quadrants read partition-shifted slices of xt)
    nc.vector.tensor_copy(TEbuf[0:64, :, 0, :], xt[0:64, :, :])
    nc.vector.tensor_copy(TEbuf[64:128, :, 1, :], xt[64:128, :, :])
    nc.vector.tensor_copy(TObuf[0:64, :, 0, :], xt[64:128, 0:PAIRS, :])
    nc.vector.tensor_copy(TObuf[64:128, :, 1, :], xt[0:64, 1:PAIRS + 1, :])

    s = 0  # first pair of current block
    for h, n in enumerate(BLOCKS):
        wt = wpool.tile([128, n * K * F], DT, name=f"wt{h}", tag=f"wt{h}")
        weng = nc.scalar if (DUAL_QUEUE and h % 2) else nc.sync
        weng.dma_start(wt[:], wd[:, ds(s * K * F, n * K * F)])

        def w_ap(jj, k):
            return wt[:, ds((jj * K + k) * F, F)]

        # compute in chunks of up to CCHUNK pairs (one PSUM bank-full)
        for c0 in range(0, n, CCHUNK):
            m = min(CCHUNK, n - c0)
            ngroups = m // 8
            accs = [pspool.tile([128, 512], DT_PS, name=f"acc{q}", tag=f"acc{q}")
                    for q in range(ngroups)]
            for j in range(8):
                for q in range(ngroups):
                    i = s + c0 + q * 8 + j   # global pair
                    jj = c0 + q * 8 + j      # pair in dma block
                    o_ap = accs[q][ds(32 * q, 16), ts(j, 64)]
                    tp = (0, 32 * q)
                    nc.tensor.matmul(o_ap, TEbuf[:, i, :, :], w_ap(jj, 0),
                                     start=True, stop=False, tile_position=tp)
                    nc.tensor.matmul(o_ap, TObuf[:, i, :, :], w_ap(jj, 1),
                                     start=False, stop=False, tile_position=tp)
                    nc.tensor.matmul(o_ap, TEbuf[:, i + 1, :, :], w_ap(jj, 2),
                                     start=False, stop=True, tile_position=tp)
            # drain PSUM (f32 -> bf16) into the output staging buffer
            g0 = (s + c0) // 8  # first global group of this chunk
            for q in range(ngroups):
                dst = obuf[:, ds((g0 + q) * 512, 512)]
                src = accs[q][ds(32 * q, 16), :]
                if DRAIN_SPLIT and q % 2:
                    nc.scalar.copy(dst, src)
                else:
                    nc.vector.tensor_copy(dst, src)
            if OUT_MODE == "block_gpsimd":
                # SWDGE path keeps the HWDGE completion-sem lanes for inputs
                nc.gpsimd.dma_start(od[:, ds(g0 * 512, ngroups * 512)],
                                    obuf[:, ds(g0 * 512, ngroups * 512)])
        s += n

    if OUT_MODE == "chunk_act":
        # chunked output DMAs, emitted after all weight DMAs so the HWDGE
        # completion-sem lanes of the input stream never wait behind outputs
        for b0, b1 in OUT_CHUNKS:
            g0 = sum(BLOCKS[:b0]) // 8
            g1 = sum(BLOCKS[:b1]) // 8
            nc.scalar.dma_start(od[:, ds(g0 * 512, (g1 - g0) * 512)],
                                obuf[:, ds(g0 * 512, (g1 - g0) * 512)])


def _build_nc(n_iters=None):
    """n_iters=None: straight-line kernel (graded path).
    n_iters=N: body wrapped in a HW For_i loop, for timing-slope runs."""
    nc = bacc.Bacc("TRN2", target_bir_lowering=False, debug=False)

    wd = nc.declare_dram_parameter("wd", [128, W_COLS], DT, isOutput=False)
    xd = nc.declare_dram_parameter("xd", [128, X_COLS], DT, isOutput=False)
    # out[m, g*512 + j*64 + f]: g = group of 8 pairs, m = phase*8 + b.
    od = nc.declare_dram_parameter("out", [16, (PAIRS // 8) * 512], DT,
                                   isOutput=True)

    with tile.TileContext(nc) as tc:
        with (
            tc.tile_pool(name="xpool", bufs=1) as xpool,
            tc.tile_pool(name="wpool", bufs=1) as wpool,
            # 4 acc tags (one per PE strip) x 2 bufs = all 8 PSUM banks
            tc.tile_pool(name="pspool", bufs=2, space=bass.MemorySpace.PSUM) as pspool,
        ):
            TEbuf = xpool.tile([128, PAIRS + 1, 2, 8], DT, name="TEbuf", tag="TEbuf")
            TObuf = xpool.tile([128, PAIRS, 2, 8], DT, name="TObuf", tag="TObuf")
            xt = xpool.tile([128, PAIRS + 1, 8], DT, name="xt", tag="xt")
            obuf = xpool.tile([16, (PAIRS // 8) * 512], DT, name="obuf", tag="obuf")
            xbufs = (TEbuf, TObuf, xt, obuf)

            # zero the off-diagonal quadrants of TE/TO once (outside the
            # timing loop; compute never overwrites them)
            nc.gpsimd.memset(TEbuf[0:64, :, 1, :], 0.0)
            nc.gpsimd.memset(TEbuf[64:128, :, 0, :], 0.0)
            nc.gpsimd.memset(TObuf[0:64, :, 1, :], 0.0)
            nc.gpsimd.memset(TObuf[64:128, :, 0, :], 0.0)

            if n_iters is None:
                _build_body(nc, wpool, pspool, xbufs, wd, xd, od)
            else:
                with tc.For_i(0, n_iters, 1):
                    _build_body(nc, wpool, pspool, xbufs, wd, xd, od)

    nc.compile()
    return nc


def _prep_inputs(x, kernel):
    """Host-side rearrangement into per-core DRAM layouts."""
    xp = np.zeros((B, L + 4, C), np.float32)
    xp[:, :L] = x
    kp = np.zeros((N_CORES * P_CORE, K, C, F), np.float32)
    kp[:L_OUT] = kernel
    in_maps = []
    for m in range(N_CORES):
        l0 = P_CORE * m
        # weights: partition (p, c), col ((pair, k), f)
        W = (kp[l0:l0 + P_CORE]
             .reshape(PAIRS, 2, K, C, F)
             .transpose(1, 3, 0, 2, 4)
             .reshape(128, W_COLS))
        # dense x: top half (c, (i, b)) = x[b, l0+2i, c]; bottom = odd pos
        xs = xp[:, l0:l0 + 2 * (PAIRS + 1), :]
        ev = xs[:, 0::2].transpose(2, 1, 0)  # (64, 257, 8)  position 2i
        od_ = xs[:, 1::2].transpose(2, 1, 0)  # (64, 257, 8)  position 2i+1
        XD = np.concatenate([ev, od_], axis=0).reshape(128, X_COLS)
        in_maps.append({"wd": W.astype(NPDT), "xd": XD.astype(NPDT)})
    return in_maps


def _unpack_out(res):
    """(16, 32*512) per core -> (B, P_CORE, F).  l_local = 16g + 2j + phase."""
    return (res.astype(np.float32)
            .reshape(2, 8, 32, 8, 64)              # [phase, b, g, j, f]
            .transpose(1, 2, 3, 0, 4)              # [b, g, j, phase, f]
            .reshape(B, P_CORE, F))


def kernel(x, kernel, bias):
    x = np.asarray(x, dtype=np.float32)
    kern = np.asarray(kernel, dtype=np.float32)
    bias = np.asarray(bias, dtype=np.float32)

    if "nc" not in _CACHE:
        _CACHE["nc"] = _build_nc()
    nc = _CACHE["nc"]

    in_maps = _prep_inputs(x, kern)
    results = run_bass_kernel_spmd(nc, in_maps, list(range(N_CORES))).results

    parts = [_unpack_out(results[m]["out"]) for m in range(N_CORES)]
    out = np.concatenate(parts, axis=1)[:, :L_OUT]
    return (out + bias[None]).astype(np.float32)


# revision 25
# speedup vs baseline: 1.1615x; 1.1615x over previous
"""LocallyConnected1D (B=8, L=4096, C=64, K=3, F=64) on 8 TRN2 NeuronCores.

out[b, l, f] = sum_{k,c} x[b, l+k, c] * kernel[l, k, c, f] + bias[l, f]

Strategy (spatial sharding, 512 output positions per core):
  - For each pair of adjacent output positions (l0+2i, l0+2i+1) build a
    block-diagonal stationary tile lhsT (128 x 16): partitions = 2 phases x 64
    channels, columns = 2 phases x 8 batch.  Streaming operand = the pair's
    per-position weights (128 x 64).  Three PSUM-accumulated matmuls per pair
    (one per tap k, using x-pair tiles shifted by k) produce out (16, 64).
  - Groups of 8 pairs are dispatched to 4 independent 32-column strips of the
    PE array (tile_position), each strip accumulating into its own PSUM bank.
  - HBM traffic is minimized (the kernel is HBM-bound): weights stream as
    bf16 blocks; x is DMA'd ONCE densely (0.5 MB) and the block-diagonal
    TE/TO stationary tiles are built on-chip with 4 strided DVE copies
    (zero quadrants pre-memset on GpSimd); outputs return as bf16.
  - Compute in bf16 (PSUM accumulation in f32); bias added on host.
"""

import numpy as np
import ml_dtypes

import concourse.bass as bass
import concourse.mybir as mybir
import concourse.tile as tile
from concourse import bacc
from concourse.bass import ds, ts
from concourse.bass_utils import run_bass_kernel_spmd

B, L, C, K, F = 8, 4096, 64, 3, 64
L_OUT = (L - K) + 1  # 4094
N_CORES = 8
P_CORE = 512          # output positions per core (last core: 510 real + 2 pad)
PAIRS = P_CORE // 2   # 256

# pairs per weight-DMA block; tapered tail lets the last blocks' compute and
# drains pipeline under the DMA stream, shrinking the post-stream tail
BLOCKS = [32, 32, 32, 32, 32, 24, 24, 16, 16, 8, 8]
assert sum(BLOCKS) == PAIRS and all(n % 8 == 0 for n in BLOCKS)
# pairs per compute chunk (one PSUM bank-full across up to 4 PE strips)
CCHUNK = 32
# output chunks (by block index): few out-DMAs, tiny final one
OUT_CHUNKS = [(0, 4), (4, 7), (7, 10), (10, 11)]
DRAIN_SPLIT = False  # alternate PSUM drains between DVE and Activation
OUT_MODE = "block_gpsimd"  # "chunk_act" | "block_gpsimd"
DUAL_QUEUE = False  # alternate weight blocks between the two HWDGE queues
XD_QUEUE = "sync"   # queue for the dense-x DMA: "sync" | "scalar"

DT = mybir.dt.bfloat16
NPDT = ml_dtypes.bfloat16
DT_PS = mybir.dt.float32

W_COLS = PAIRS * K * F           # 49152 weight cols (192 per pair)
X_COLS = (PAIRS + 1) * 8         # 2056 dense-x cols

_CACHE = {}


def _build_body(nc, wpool, pspool, xbufs, wd, xd, od):
    TEbuf, TObuf, xt, obuf = xbufs

    # dense x in; same queue as the weight stream avoids inter-queue
    # packet interleave on the SDMA engines
    (nc.sync if XD_QUEUE == "sync" else nc.scalar).dma_start(xt[:], xd[:])

    # scatter dense x into the diagonal quadrants (strided DVE copies;
    # TO's quadrants read partition-shifted slices of xt)
    nc.vector.tensor_copy(TEbuf[0:64, :, 0, :], xt[0:64, :, :])
    nc.vector.tensor_copy(TEbuf[64:128, :, 1, :], xt[64:128, :, :])
    nc.vector.tensor_copy(TObuf[0:64, :, 0, :], xt[64:128, 0:PAIRS, :])
    nc.vector.tensor_copy(TObuf[64:128, :, 1, :], xt[0:64, 1:PAIRS + 1, :])

    s = 0  # first pair of current block
    for h, n in enumerate(BLOCKS):
        wt = wpool.tile([128, n * K * F], DT, name=f"wt{h}", tag=f"wt{h}")
        weng = nc.scalar if (DUAL_QUEUE and h % 2) else nc.sync
        weng.dma_start(wt[:], wd[:, ds(s * K * F, n * K * F)])

        def w_ap(jj, k):
            return wt[:, ds((jj * K + k) * F, F)]

        # compute in chunks of up to CCHUNK pairs (one PSUM bank-full)
        for c0 in range(0, n, CCHUNK):
            m = min(CCHUNK, n - c0)
            ngroups = m // 8
            accs = [pspool.tile([128, 512], DT_PS, name=f"acc{q}", tag=f"acc{q}")
                    for q in range(ngroups)]
            for j in range(8):
                for q in range(ngroups):
                    i = s + c0 + q * 8 + j   # global pair
                    jj = c0 + q * 8 + j      # pair in dma block
                    o_ap = accs[q][ds(32 * q, 16), ts(j, 64)]
                    tp = (0, 32 * q)
                    nc.tensor.matmul(o_ap, TEbuf[:, i, :, :], w_ap(jj, 0),
                                     start=True, stop=False, tile_position=tp)
                    nc.tensor.matmul(o_ap, TObuf[:, i, :, :], w_ap(jj, 1),
                                     start=False, stop=False, tile_position=tp)
                    nc.tensor.matmul(o_ap, TEbuf[:, i + 1, :, :], w_ap(jj, 2),
                                     start=False, stop=True, tile_position=tp)
            # drain PSUM (f32 -> bf16) into the output staging buffer
            g0 = (s + c0) // 8  # first global group of this chunk
            for q in range(ngroups):
                dst = obuf[:, ds((g0 + q) * 512, 512)]
                src = accs[q][ds(32 * q, 16), :]
                if DRAIN_SPLIT and q % 2:
                    nc.scalar.copy(dst, src)
                else:
                    nc.vector.tensor_copy(dst, src)
            if OUT_MODE == "block_gpsimd":
                # SWDGE path keeps the HWDGE completion-sem lanes for inputs
                nc.gpsimd.dma_start(od[:, ds(g0 * 512, ngroups * 512)],
                                    obuf[:, ds(g0 * 512, ngroups * 512)])
        s += n

    if OUT_MODE == "chunk_act":
        # chunked output DMAs, emitted after all weight DMAs so the HWDGE
        # completion-sem lanes of the input stream never wait behind outputs
        for b0, b1 in OUT_CHUNKS:
            g0 = sum(BLOCKS[:b0]) // 8
            g1 = sum(BLOCKS[:b1]) // 8
            nc.scalar.dma_start(od[:, ds(g0 * 512, (g1 - g0) * 512)],
                                obuf[:, ds(g0 * 512, (g1 - g0) * 512)])


def _build_nc(n_iters=None):
    """n_iters=None: straight-line kernel (graded path).
    n_iters=N: body wrapped in a HW For_i loop, for timing-slope runs."""
    nc = bacc.Bacc("TRN2", target_bir_lowering=False, debug=False)

    wd = nc.declare_dram_parameter("wd", [128, W_COLS], DT, isOutput=False)
    xd = nc.declare_dram_parameter("xd", [128, X_COLS], DT, isOutput=False)
    # out[m, g*512 + j*64 + f]: g = group of 8 pairs, m = phase*8 + b.
    od = nc.declare_dram_parameter("out", [16, (PAIRS // 8) * 512], DT,
                                   isOutput=True)

    with tile.TileContext(nc) as tc:
        with (
            tc.tile_pool(name="xpool", bufs=1) as xpool,
            tc.tile_pool(name="wpool", bufs=1) as wpool,
            # 4 acc tags (one per PE strip) x 2 bufs = all 8 PSUM banks
            tc.tile_pool(name="pspool", bufs=2, space=bass.MemorySpace.PSUM) as pspool,
        ):
            TEbuf = xpool.tile([128, PAIRS + 1, 2, 8], DT, name="TEbuf", tag="TEbuf")
            TObuf = xpool.tile([128, PAIRS, 2, 8], DT, name="TObuf", tag="TObuf")
            xt = xpool.tile([128, PAIRS + 1, 8], DT, name="xt", tag="xt")
            obuf = xpool.tile([16, (PAIRS // 8) * 512], DT, name="obuf", tag="obuf")
            xbufs = (TEbuf, TObuf, xt, obuf)

            # zero the off-diagonal quadrants of TE/TO once (outside the
            # timing loop; compute never overwrites them)
            nc.gpsimd.memset(TEbuf[0:64, :, 1, :], 0.0)
            nc.gpsimd.memset(TEbuf[64:128, :, 0, :], 0.0)
            nc.gpsimd.memset(TObuf[0:64, :, 1, :], 0.0)
            nc.gpsimd.memset(TObuf[64:128, :, 0, :], 0.0)

            if n_iters is None:
                _build_body(nc, wpool, pspool, xbufs, wd, xd, od)
            else:
                with tc.For_i(0, n_iters, 1):
                    _build_body(nc, wpool, pspool, xbufs, wd, xd, od)

    nc.compile()
    return nc


def _prep_inputs(x, kernel):
    """Host-side rearrangement into per-core DRAM layouts."""
    xp = np.zeros((B, L + 4, C), np.float32)
    xp[:, :L] = x
    kp = np.zeros((N_CORES * P_CORE, K, C, F), np.float32)
    kp[:L_OUT] = kernel
    in_maps = []
    for m in range(N_CORES):
        l0 = P_CORE * m
        # weights: partition (p, c), col ((pair, k), f)
        W = (kp[l0:l0 + P_CORE]
             .reshape(PAIRS, 2, K, C, F)
             .transpose(1, 3, 0, 2, 4)
             .reshape(128, W_COLS))
        # dense x: top half (c, (i, b)) = x[b, l0+2i, c]; bottom = odd pos
        xs = xp[:, l0:l0 + 2 * (PAIRS + 1), :]
        ev = xs[:, 0::2].transpose(2, 1, 0)  # (64, 257, 8)  position 2i
        od_ = xs[:, 1::2].transpose(2, 1, 0)  # (64, 257, 8)  position 2i+1
        XD = np.concatenate([ev, od_], axis=0).reshape(128, X_COLS)
        in_maps.append({"wd": W.astype(NPDT), "xd": XD.astype(NPDT)})
    return in_maps


def _unpack_out(res):
    """(16, 32*512) per core -> (B, P_CORE, F).  l_local = 16g + 2j + phase."""
    return (res.astype(np.float32)
            .reshape(2, 8, 32, 8, 64)              # [phase, b, g, j, f]
            .transpose(1, 2, 3, 0, 4)              # [b, g, j, phase, f]
            .reshape(B, P_CORE, F))


def kernel(x, kernel, bias):
    x = np.asarray(x, dtype=np.float32)
    kern = np.asarray(kernel, dtype=np.float32)
    bias = np.asarray(bias, dtype=np.float32)

    if "nc" not in _CACHE:
        _CACHE["nc"] = _build_nc()
    nc = _CACHE["nc"]

    in_maps = _prep_inputs(x, kern)
    results = run_bass_kernel_spmd(nc, in_maps, list(range(N_CORES))).results

    parts = [_unpack_out(results[m]["out"]) for m in range(N_CORES)]
    out = np.concatenate(parts, axis=1)[:, :L_OUT]
    return (out + bias[None]).astype(np.float32)
